# revision 64
# baseline (speedup 1.0000x reference)
"""CondConv2d on 8 Trainium2 NeuronCores — data-parallel over batch N=8.

Per-core (one sample):
  - TWO copies of x stream from HBM on one ring: a small fp8(e3m4) copy
    first (it gates the whole attention branch ~2x earlier than bf16
    would), then the bf16 conv copy whose row-shifted upper duplicate
    (partitions 64-127) is made by SBUF->SBUF DMAs chunk-by-chunk, with
    the mixing banks interleaved in consumption order.  Chunk sizes are
    staggered because concurrently-pending DMAs round-robin descriptors
    across the 16 queues (trigger order does NOT give priority).
  - The attention branch (three global-mean-pooled conv3ds) collapses to
    a linear function of basis sums of x, computed entirely from the fp8
    copy and split across three engines as chunks land: PE selector
    matmuls (edge-channel + grand totals via a [4+1]-row selector into
    one PSUM group), DVE folds, and ACT identity-accumulate spans, all
    landing in one [64, NBASIS] basis matrix; a fused 4-op DVE
    contraction + one tiny matmul produce the logits.
  - Softmax normalization is skipped: weights are mixed (bf16 DVE ops)
    with raw exp(logits) — the static conv_w bank is pre-folded into
    every bank host-side since sum(att)=1 — and the 1/sum scale is
    applied at PSUM eviction together with the conv bias.
  - The 3x3 conv runs even/odd-row interleaved at 75% PE utilization:
    PSUM partitions = 64 out-ch x {even,odd} rows, contraction 128 =
    64 in-ch x {row r, row r+1} (the shifted upper copy).  6 matmuls
    (2 row-bases x 3 width taps) of [128x128x390] cover SIX output rows
    (3 row-pairs) per PSUM tile — 22 tiles, half the PE cycles of a
    64-wide layout.  The rhs is a strided (pair, 130) view of XL.
  - Evictions strip the pad columns into one contiguous bf16 staging
    buffer; output DMAs go out in 4-tile groups (3KB descriptors) on two
    rings, and the host deinterleaves the parity-major layout.
  - Keep-warm filler matmuls bridge PE idle windows so its p-state stays
    up into the conv.
"""
import numpy as np

CONV_DT = "bf16"
N, C, H, W = 8, 64, 128, 128
K = 4
WP = W + 2                 # padded row width (130)
NELEM = WP * H + 262       # per-partition x buffer length (16902)
JT = 3                     # row-pairs per conv tile (6 output rows)
NPAIR = H // 2
GS = 5                     # conv tiles in the m-outer head group
NPSB = 6                   # conv PSUM tile buffers
NT = (NPAIR + JT - 1) // JT  # 22

# fp8 attention copy of x, stored [128, 8320]: partitions 0-63 = image rows
# 0-64, partitions 64-127 = rows 64-128 -> 128-contraction selector matmuls
# absorb 1024 elems each.  Column-chunks staggered (spray scheduling).
XHALF = WP * 64            # 8320
XCHUNK_COLS = [1560, 2080, 2080, 2600]
XCHUNK_OFF = list(np.cumsum([0] + XCHUNK_COLS))
# bf16 conv copy of x: chunks sized to stay ahead of the conv wavefront
BCHUNK_ROWS = [16, 24, 32, 32, 24]
BCHUNK_OFF = list(np.cumsum([0] + BCHUNK_ROWS))

# conv matmul configs: (row base b, width tap dw)
BCFG = [(-1, -1), (-1, 0), (-1, 1), (1, -1), (1, 0), (1, 1)]

NBASIS = 17
SMM_W = 512                # selector-matmul free width
NSMM = 13                  # selector covers cols [0, 6656) of BOTH halves
# DVE fold strips (both partition halves at once) for the remainder
FOLD_SPANS = [(6656, 7488), (7488, 8320)]


# ----------------------------------------------------------------------------
# host-side prep
# ----------------------------------------------------------------------------
def _make_cw(net0_w, net0_b, net1_w, net1_b, net2_w, net2_b):
    """CW[c, b, k] over the 10 logical bases:
    0=total, 1=row0, 2=row127, 3=col0, 4=col127,
    5..8=corners (00,0W,H0,HW), 9=const 1."""
    cw = np.zeros((C, 10, K), np.float64)
    scale = 1.0 / (C * H * W)
    for w_net, pads in ((net0_w, (0, 0, 0)), (net1_w, (1, 1, 1)), (net2_w, (2, 1, 1))):
        Kk, _, kd, kh, kw = w_net.shape
        pd, ph, pw = pads
        for i in range(kd):
            clo, chi = max(0, i - pd), min(C - 1, C - 1 + i - pd)
            cmask = np.zeros(C)
            cmask[clo:chi + 1] = 1.0
            for j in range(kh):
                hlo, hhi = max(0, j - ph), min(H - 1, H - 1 + j - ph)
                dropA = 0 if hlo == 1 else (127 if hhi == H - 2 else None)
                for l in range(kw):
                    wlo, whi = max(0, l - pw), min(W - 1, W - 1 + l - pw)
                    dropB = 0 if wlo == 1 else (127 if whi == W - 2 else None)
                    v = np.zeros(10)
                    v[0] = 1.0
                    if dropA == 0: v[1] = -1.0
                    if dropA == 127: v[2] = -1.0
                    if dropB == 0: v[3] = -1.0
                    if dropB == 127: v[4] = -1.0
                    if dropA is not None and dropB is not None:
                        v[{(0, 0): 5, (0, 127): 6, (127, 0): 7, (127, 127): 8}[(dropA, dropB)]] = 1.0
                    for k in range(Kk):
                        cw[:, :, k] += w_net[k, 0, i, j, l] * scale * np.outer(cmask, v)
    btot = (net0_b + net1_b + net2_b).astype(np.float64)
    cw[:, 9, :] += btot[None, :] / C
    return cw


EDGE_CH = [0, 1, 62, 63]


def _make_cw2(cw):
    """Expand CW (C,10,K) to [128, NBASIS, K]; rows 0-63 weigh lower-half
    (image rows 0-64) sums, rows 64-127 upper-half.  Cols: 0 selector PSUM
    (partitions 0-3 edge partials, 4 grand), 1 row0(lo), 2 row127(up),
    3/4/5 col0 parts (lo, up-a, up-b), 6/7/8 col127 parts, 9/10 row-0
    corners (lo), 11/12 row-127 corners (up; absorb row-127 col values),
    13 const(lo), 14/15 strip folds (both), 16 lo row-63 col-127 value
    (the lo col127 part stops at row 62 to stay in-slice)."""
    cwmid = cw[C // 2, 0, :]
    assert np.abs(cw[2:62, 0, :] - cwmid[None, :]).max() < 1e-12
    cwx = np.zeros((128, NBASIS, K), np.float64)
    lo, up = cwx[0:64], cwx[64:128]
    for i, e in enumerate(EDGE_CH):
        cwx[i, 0, :] = cw[e, 0, :] - cwmid
    cwx[4, 0, :] = cwmid
    lo[:, 1] = cw[:, 1]
    up[:, 2] = cw[:, 2]
    lo[:, 3] = cw[:, 3]; up[:, 4] = cw[:, 3]; up[:, 5] = cw[:, 3]
    lo[:, 6] = cw[:, 4]; up[:, 7] = cw[:, 4]; up[:, 8] = cw[:, 4]
    lo[:, 9] = cw[:, 5]; lo[:, 10] = cw[:, 6]
    up[:, 11] = cw[:, 7] + cw[:, 3]
    up[:, 12] = cw[:, 8] + cw[:, 4]
    lo[:, 13] = cw[:, 9]
    lo[:, 14] = cw[:, 0]; up[:, 14] = cw[:, 0]
    lo[:, 15] = cw[:, 0]; up[:, 15] = cw[:, 0]
    lo[:, 16] = cw[:, 4]
    return np.ascontiguousarray(cwx.astype(np.float32))


def _make_bank(Wt):
    """Wt (co, ci, 3, 3) -> (128, 6, 128): [p=(shift s, ci), m=(b,dw),
    (parity, co)].  Block [s][par] holds W[:, :, 1 + rowtap, 1 + dw].T where
    rowtap = (b + s) - par; invalid taps are zero."""
    bank = np.zeros((128, 6, 128), np.float32)
    for m, (b, dw) in enumerate(BCFG):
        for s in (0, 1):
            for par in (0, 1):
                rt = b + s - par
                if -1 <= rt <= 1:
                    bank[s * 64:s * 64 + 64, m, par * 64:par * 64 + 64] = \
                        Wt[:, :, 1 + rt, 1 + dw].T
    return bank


# ----------------------------------------------------------------------------
# device program
# ----------------------------------------------------------------------------
_NC_CACHE = {}


def _build_nc():
    import concourse.bacc as bacc
    import concourse.tile as tile
    from concourse import mybir

    f32 = mybir.dt.float32
    DT = mybir.dt.bfloat16
    Alu = mybir.AluOpType
    Ax = mybir.AxisListType
    Act = mybir.ActivationFunctionType

    F8 = mybir.dt.float8e3

    nc = bacc.Bacc("TRN2", target_bir_lowering=False, debug=False,
                   enable_asserts=False, num_devices=N)
    xin = nc.dram_tensor("xin", [C, H * WP], DT, kind="ExternalInput")
    xf8d = nc.dram_tensor("xf8", [128, XHALF], F8, kind="ExternalInput")
    seld = nc.dram_tensor("sel", [128, 128], F8, kind="ExternalInput")
    wbk = nc.dram_tensor("wbanks", [128, 6, K, 128], DT, kind="ExternalInput")
    cw2 = nc.dram_tensor("cw2", [128, NBASIS, K], f32, kind="ExternalInput")
    cb = nc.dram_tensor("convb", [128, 1], f32, kind="ExternalInput")
    # output partition p = parity*64 + channel; rows of one parity are
    # contiguous per partition so each eviction DMA is one 768B descriptor
    # per partition (host deinterleaves)
    outT = nc.dram_tensor("out", [128, NPAIR, W], DT, kind="ExternalOutput")

    with tile.TileContext(nc) as tc:
        with tc.tile_pool(name="singles", bufs=1) as S, \
             tc.tile_pool(name="spsum", bufs=1, space="PSUM") as PS1, \
             tc.tile_pool(name="cpsum", bufs=NPSB, space="PSUM") as PS:

            XL = S.tile([128, NELEM], DT)
            XF = S.tile([128, XHALF], F8)
            wb_sb = S.tile([128, 6, K, 128], DT)
            cw2_sb = S.tile([128, NBASIS, K], f32)
            convb_sb = S.tile([128, 1], f32)
            zlhs = S.tile([128, 128], DT)
            sel = S.tile([128, 128], F8)
            onesall = S.tile([128, 128], DT)
            att_sb = S.tile([128, K], f32)
            M = S.tile([128, NBASIS], f32)
            G = S.tile([128, K], f32)
            Gb = S.tile([128, K], DT)
            gscr = S.tile([128, NBASIS], f32)
            mw = S.tile([128, 6, 128], DT)
            mwb = S.tile([128, 6, 128], DT)
            actscr = S.tile([128, 2080], f32)
            foldA = S.tile([128, 832], DT)
            foldB = S.tile([128, 832], DT)
            SG = S.tile([128, NPAIR * W], DT)
            ssum = S.tile([128, 1], f32)
            sinv = S.tile([128, 1], f32)
            scr2 = S.tile([64, 4], F8)

            psum_s = PS1.tile([128, SMM_W], f32)
            wpsum = PS1.tile([128, 512], f32)
            # logits land in a spare corner of the filler bank (frees a PSUM
            # bank for a 6th conv tile buffer)
            psum_b = wpsum[:, 480:480 + K]

            # --- constants / border zeroing (all tiny) ---
            nc.vector.memset(zlhs, 0.0)
            nc.vector.memset(onesall, 1.0)
            nc.vector.memset(M, 0.0)
            nc.vector.memset(M[:, 13:14], 1.0)
            # borders: host pre-pads the row gaps; only head/tail need zeroing
            nc.vector.memset(XL[0:64, 0:132], 0.0)
            nc.vector.memset(XL[0:64, 132 + H * WP:NELEM], 0.0)
            nc.vector.memset(XL[64:128, 0:2], 0.0)
            nc.vector.memset(XL[64:128, 2 + H * WP:NELEM], 0.0)

            # --- PE pipeline warm-up (results discarded; zlhs is all-zero) ---
            for i in range(8):
                nc.tensor.matmul(wpsum[:, 0:128], zlhs, zlhs, start=True, stop=True)

            # --- input DMAs, all on one ring so queue order is exact:
            # the small fp8 attention copy of x loads FIRST (it gates the
            # whole attention pipeline), then mixing banks and the bf16 conv
            # copy + its SBUF->SBUF row-shifted upper copies, interleaved in
            # conv-consumption order.  Output DMAs live on other rings.
            nc.scalar.dma_start(out=sel, in_=seld[:, :])
            nc.scalar.dma_start(out=cw2_sb, in_=cw2[:, :, :])
            nc.scalar.dma_start(out=convb_sb, in_=cb[:, :])
            for c in range(len(XCHUNK_COLS)):
                a, b = XCHUNK_OFF[c], XCHUNK_OFF[c + 1]
                nc.sync.dma_start(out=XF[:, a:b], in_=xf8d[:, a:b])
            # two dummy DMAs occupy ring-depth slots so the bank/bf16 stream
            # below cannot spray descriptors into the fp8 load's tail (the
            # DMA ring admits ~5 in-flight; slot 6+ waits on completions)
            nc.sync.dma_start(out=scr2[:, 0:2], in_=xf8d[0:64, 0:2])
            nc.sync.dma_start(out=scr2[:, 2:4], in_=xf8d[0:64, 2:4])

            def bchunk(c):
                a = WP * BCHUNK_OFF[c]
                ln = WP * BCHUNK_ROWS[c]
                nc.sync.dma_start(out=XL[0:64, 132 + a: 132 + a + ln],
                                  in_=xin[:, a: a + ln])
                nc.sync.dma_start(out=XL[64:128, 2 + a: 2 + a + ln],
                                  in_=XL[0:64, 132 + a: 132 + a + ln])

            nc.sync.dma_start(out=wb_sb[:, 0:2, :, :], in_=wbk[:, 0:2, :, :])
            bchunk(0)
            bchunk(1)
            nc.sync.dma_start(out=wb_sb[:, 2:6, :, :], in_=wbk[:, 2:6, :, :])
            for c in range(2, len(BCHUNK_ROWS)):
                bchunk(c)

            # --- totals: 128-contraction selector matmuls absorb both
            # image halves at once; DVE/ACT cover the trailing strips and
            # row/col/corner sums (lo = partitions 0-63, up = 64-127).
            for j in range(NSMM):
                a = SMM_W * j
                nc.tensor.matmul(psum_s, sel, XF[:, a:a + SMM_W],
                                 start=(j == 0), stop=(j == NSMM - 1))

            def warm(n):
                # p-state fillers on early-landing data (results discarded)
                for i in range(n):
                    nc.tensor.matmul(wpsum[:, 0:512], sel, XF[:, 2560:3072],
                                     start=True, stop=True)

            lo, up = XF[0:64], XF[64:128]

            def colpart(half, mlo, col, u0, u1, mcol, eng):
                a = WP * u0 + col
                v = half[:, a:a + WP * (u1 - u0)].rearrange(
                    "p (r w) -> p r w", w=WP)[:, :, 0:1]
                if eng is nc.vector:
                    nc.vector.tensor_reduce(out=M[mlo:mlo + 64, mcol:mcol + 1],
                                            in_=v, axis=Ax.XY, op=Alu.add)
                else:
                    nc.scalar.activation(
                        out=actscr[mlo:mlo + 64, 0:u1 - u0].rearrange(
                            "p (r w) -> p r w", w=1), in_=v,
                        func=Act.Identity,
                        accum_out=M[mlo:mlo + 64, mcol:mcol + 1])

            def fold(i, obuf):
                a, b = FOLD_SPANS[i]
                h = (b - a) // 2
                nc.vector.scalar_tensor_tensor(
                    out=obuf[:, :h], in0=XF[:, a:a + h], scalar=1.0,
                    in1=XF[:, a + h:b], op0=Alu.mult, op1=Alu.add,
                    accum_out=M[:, 14 + i:15 + i])

            # DVE: row0 + row-0 corners early, then the last-chunk tail
            nc.vector.tensor_reduce(out=M[0:64, 1:2], in_=lo[:, 0:W],
                                    axis=Ax.X, op=Alu.add)
            nc.vector.tensor_copy(
                out=M[0:64, 9:11].rearrange("p (a b) -> p a b", b=1),
                in_=lo[:, 0:254].rearrange("p (a b) -> p a b", b=127)[:, :, 0:1])
            fold(0, foldA)
            fold(1, foldB)
            colpart(up, 64, 0, 0, 48, 4, nc.vector)
            colpart(up, 64, 0, 48, 63, 5, nc.vector)
            nc.vector.tensor_reduce(out=M[64:128, 2:3], in_=up[:, 8190:8318],
                                    axis=Ax.X, op=Alu.add)
            nc.vector.tensor_copy(out=M[64:128, 11:12], in_=up[:, 8190:8191])
            nc.vector.tensor_copy(out=M[64:128, 12:13], in_=up[:, 8317:8318])
            nc.vector.tensor_copy(out=M[0:64, 16:17], in_=lo[:, 8317:8318])
            # ACT: selector-PSUM reduce + the remaining col parts
            nc.scalar.activation(out=actscr[0:64, 0:SMM_W], in_=psum_s[0:64, :],
                                 func=Act.Identity, accum_out=M[0:64, 0:1])
            colpart(lo, 0, 0, 0, 64, 3, nc.scalar)
            colpart(lo, 0, 127, 0, 63, 6, nc.scalar)
            colpart(up, 64, 127, 0, 48, 7, nc.scalar)
            colpart(up, 64, 127, 48, 63, 8, nc.scalar)

            # per-channel coefficient contraction: G[c,k] = sum_b M[c,b]*CW2[c,b,k]
            for k in range(K):
                nc.vector.scalar_tensor_tensor(
                    out=gscr, in0=M, scalar=1.0,
                    in1=cw2_sb[:, :, k], op0=Alu.mult, op1=Alu.mult,
                    accum_out=G[:, k:k + 1])

            # keep-warm fillers: PE issue is in-order, so the first batch
            # runs from selector end until the logits are ready; the second
            # batch bridges softmax+mixing into the conv.
            warm(9)
            # logits broadcast to all 128 partitions with one bf16 matmul
            nc.vector.tensor_copy(out=Gb, in_=G)
            nc.tensor.matmul(psum_b, onesall, Gb, start=True, stop=True)
            # unnormalized softmax: att = exp(logits); 1/sum applied at eviction
            nc.scalar.activation(out=att_sb, in_=psum_b, func=Act.Exp)
            warm(2)

            # --- weight mixing: mwb[:,m,:] = sum_k exp_k * bank'_k[:,m,:]
            # (all-bf16 so the DVE runs at its 16-bit 2x rate) ---
            def mixbank(m):
                nc.vector.tensor_scalar_mul(out=mw[:, m, :], in0=wb_sb[:, m, 0, :],
                                            scalar1=att_sb[:, 0:1])
                for k in range(1, K):
                    tgt = mwb if k == K - 1 else mw
                    nc.vector.scalar_tensor_tensor(
                        out=tgt[:, m, :], in0=wb_sb[:, m, k, :],
                        scalar=att_sb[:, k:k + 1], in1=mw[:, m, :],
                        op0=Alu.mult, op1=Alu.add)

            mixbank(0)
            mixbank(1)
            for m in range(2, 6):
                mixbank(m)
            nc.vector.tensor_reduce(out=ssum, in_=att_sb, axis=Ax.X, op=Alu.add)
            nc.vector.reciprocal(out=sinv, in_=ssum)

            # --- main conv (even/odd interleaved, 6 rows per tile) ---
            def tile_jt(t):
                return min(JT, NPAIR - JT * t)

            def mktile(t):
                return PS.tile([128, WP * JT], f32,
                               tag="cps", name=f"cps{t}")[:, :WP * tile_jt(t)]

            def conv_mm(t, pt, m):
                b, dw = BCFG[m]
                j0 = JT * t
                jt = tile_jt(t)
                s0 = 132 + b * WP + dw - 1 + 2 * WP * j0
                rhs = XL[:, s0:s0 + 2 * WP * jt].rearrange(
                    "p (j w) -> p j w", w=2 * WP)[:, :, 0:WP]
                nc.tensor.matmul(pt, mwb[:, m, :], rhs,
                                 start=(m == 0), stop=(m == 5))

            def evict(t, pt):
                jt = tile_jt(t)
                j0 = JT * t
                # strip the pad columns here (strided read of PSUM) into one
                # big contiguous staging buffer; DMAs go out in 4-tile
                # groups so each descriptor is a 3KB contiguous run
                pv = pt.rearrange("p (j w) -> p j w", w=WP)[:, :, 1:1 + W]
                sv = SG[:, W * j0:W * (j0 + jt)].rearrange(
                    "p (j w) -> p j w", w=W)
                if t % 2 == 0:
                    nc.scalar.activation(out=sv, in_=pv,
                                         func=Act.Identity,
                                         bias=convb_sb[:, 0:1],
                                         scale=sinv[:, 0:1])
                else:
                    nc.vector.tensor_scalar(
                        out=sv, in0=pv,
                        scalar1=sinv[:, 0:1],
                        scalar2=convb_sb[:, 0:1],
                        op0=Alu.mult, op1=Alu.add)
                if t in (3, 7, 11, 15, 19, NT - 1):
                    p0 = JT * (t - 3) if t != NT - 1 else JT * 20
                    p1 = JT * t + jt
                    eng = nc.scalar if (t // 4) % 2 == 0 else nc.gpsimd
                    eng.dma_start(
                        out=outT[:, p0:p1, :],
                        in_=SG[:, W * p0:W * p1].rearrange("p (j w) -> p j w", w=W))

            # head group m-outer: the first matmuls only need mixed bank 0
            pts = {t: mktile(t) for t in range(GS)}
            for m in range(6):
                for t in range(GS):
                    conv_mm(t, pts[t], m)
            for t in range(GS):
                evict(t, pts[t])
            # remaining tiles tile-major: evictions + output DMAs pipeline
            for t in range(GS, NT):
                pt = mktile(t)
                for m in range(6):
                    conv_mm(t, pt, m)
                evict(t, pt)

    nc.compile()
    return nc


def _get_nc():
    if "nc" not in _NC_CACHE:
        _NC_CACHE["nc"] = _build_nc()
    return _NC_CACHE["nc"]


def _prep_inputs(x, weight, conv_w, conv_b, net0_w, net0_b, net1_w, net1_b,
                 net2_w, net2_b):
    import ml_dtypes
    cw = _make_cw(np.asarray(net0_w, np.float32), np.asarray(net0_b, np.float32),
                  np.asarray(net1_w, np.float32), np.asarray(net1_b, np.float32),
                  np.asarray(net2_w, np.float32), np.asarray(net2_b, np.float32))
    cw2 = _make_cw2(cw)
    wf = np.asarray(weight, np.float32)
    cwf = np.asarray(conv_w, np.float32)
    # fold the static conv bank into every mixed bank (sum(att) == 1)
    banks = np.stack([_make_bank(wf[k] + cwf) for k in range(K)])  # (K,128,6,128)
    banks = np.ascontiguousarray(
        banks.transpose(1, 2, 0, 3)).astype(ml_dtypes.bfloat16)    # (128,6,K,128)
    convb = np.ascontiguousarray(
        np.tile(np.asarray(conv_b, np.float32), 2).reshape(128, 1))
    selh = np.zeros((C, 128), np.float32)
    for i, e in enumerate(EDGE_CH):
        selh[e, i] = 1.0
    selh[:, 4] = 1.0
    selh = np.ascontiguousarray(
        np.concatenate([selh, selh], axis=0).astype(ml_dtypes.float8_e3m4))
    x = np.asarray(x, np.float32)
    xp = np.zeros((N, C, H, WP), np.float32)
    xp[:, :, :, :W] = x
    xs = xp.astype(ml_dtypes.bfloat16)
    xf = xp.astype(ml_dtypes.float8_e3m4)
    in_maps = []
    for n in range(N):
        in_maps.append({
            "xin": np.ascontiguousarray(xs[n].reshape(C, H * WP)),
            "xf8": np.ascontiguousarray(np.concatenate(
                [xf[n, :, 0:64], xf[n, :, 64:128]], axis=0).reshape(128, XHALF)),
            "sel": selh,
            "wbanks": banks,
            "cw2": cw2,
            "convb": convb,
        })
    return in_maps


def _run(inputs, trace=False, **kw):
    from concourse.bass_utils import run_bass_kernel_spmd
    nc = _get_nc()
    in_maps = _prep_inputs(**inputs)
    return run_bass_kernel_spmd(nc, in_maps, core_ids=list(range(N)), trace=trace, **kw)


def _gather(res):
    # out is [parity*64+c, H/2, W] bf16 -> (C, H, W) with rows deinterleaved
    return np.stack([np.asarray(res.results[n]["out"])
                     .reshape(2, C, NPAIR, W).transpose(1, 2, 0, 3)
                     .reshape(C, H, W)
                     for n in range(N)]).astype(np.float32)


def kernel(**inputs):
    return _gather(_run(inputs))
